# revision 17
# baseline (speedup 1.0000x reference)
"""Trainium2 Bass kernel for sparse-conv (kernel-map gather-GEMM-scatter).

Math: out[j, d] = sum over points i with out_idx[i]==j of  x[i, :] @ W[k_idx[i], :, d]

Strategy ("exact k-set class GEMMs", zero-free packing):
  Each output voxel j owns the set S_j of kernel offsets its points occupy.
  Voxels are grouped into classes by that exact set (voxels with |S|>4 are
  split into two stream-voxels of <=4 offsets each; the host adds the two
  halves back together).  For a class with set S (|S|=c), the per-voxel
  contribution is a single K=32c GEMM against the stacked weight
  [W[S_0]; ...; W[S_{c-1}]]  -- so the moving operand contains ONLY real
  point data (no dense 8-slot zero fill; ~2.6x less DMA).

  Device layout: class rectangles (height 32c, width = #voxels of that class
  on this core) are shelf-packed into a single [128, F] bf16 slab; a class at
  row offset 32r runs as matmul tile_position=(32r, 32g) writing PSUM
  partitions [32g, 32g+32) -- 4 column-groups pack 4x512 voxel outputs into
  one [128, 512] PSUM bank.  PSUM -> SBUF (bf16 cast) on DVE/ACT, SWDGE DMA
  out on GpSimd.

  Stream-voxels are dealt round-robin across the 8 cores so every core has an
  identical class-count vector => one SPMD program for all cores.  The
  program structure depends only on the per-class counts (cached by key).
"""
import sys

if "/opt/trn_rl_repo" not in sys.path:
    sys.path.insert(0, "/opt/trn_rl_repo")

import numpy as np

N_CORES = 8
PSUM_N = 512          # psum bank columns (f32)
TILE_SLOTS = 2048     # 4 col-groups x 512 per psum tile
STAGE_SLOTS = 8192    # 4 psum tiles per [128, 2048] staging tile
CHUNK_COLS = 3072     # xslab DMA chunk granularity (columns)
DT_IN = "bfloat16"    # kept for test.py compat

# ---------------------------------------------------------------- mask tables
_PC = np.array([bin(m).count("1") for m in range(256)], dtype=np.int64)


def _split_mask(m):
    if _PC[m] <= 4:
        return m, 0
    bits = [b for b in range(8) if (m >> b) & 1]
    a = sum(1 << b for b in bits[:4])
    return a, m - a


_AMASK = np.array([_split_mask(m)[0] for m in range(256)], dtype=np.int64)
_BMASK = np.array([_split_mask(m)[1] for m in range(256)], dtype=np.int64)
_RANK = np.zeros((256, 8), dtype=np.int64)
for _m in range(256):
    _r = 0
    for _k in range(8):
        if (_m >> _k) & 1:
            _RANK[_m, _k] = _r
            _r += 1

_prog_cache = {}


# ------------------------------------------------------------------- planning
def _plan(x, W, k_idx, out_idx, num_out):
    """Returns (layout, meta, xslabs, wslab). layout drives program build."""
    n = x.shape[0]
    vox = out_idx.astype(np.int64)
    kk = k_idx.astype(np.int64)
    xv = x

    # guard out-of-range (reference segment_sum drops them)
    if vox.max(initial=0) >= num_out or vox.min(initial=0) < 0:
        keep = (vox >= 0) & (vox < num_out)
        vox, kk, xv = vox[keep], kk[keep], xv[keep]

    # pre-accumulate duplicate (voxel, offset) pairs
    pair = vox * 8 + kk
    order0 = np.argsort(pair, kind="stable")
    ps_sorted = pair[order0]
    uniq = np.ones(len(ps_sorted), bool)
    if len(ps_sorted) > 1:
        uniq[1:] = ps_sorted[1:] != ps_sorted[:-1]
    if not uniq.all():
        grp = np.cumsum(uniq) - 1
        m = int(grp[-1]) + 1
        xa = np.zeros((m, xv.shape[1]), np.float32)
        np.add.at(xa, grp, xv[order0])
        pu = ps_sorted[uniq]
        vox, kk, xv = pu // 8, pu % 8, xa

    # per-voxel k-set bitmask (sums of distinct powers of two == OR)
    mask = np.bincount(vox, weights=2.0 ** kk,
                       minlength=num_out).astype(np.int64)
    amask = _AMASK[mask]
    bmask = _BMASK[mask]

    avox = np.nonzero(mask)[0]
    bvox = np.nonzero(bmask)[0]
    sv_vox = np.concatenate([avox, bvox])
    sv_mask = np.concatenate([amask[avox], bmask[bvox]])
    sv_isb = np.concatenate([np.zeros(len(avox), np.int64),
                             np.ones(len(bvox), np.int64)])
    nsv = len(sv_vox)

    svo = np.argsort(sv_mask, kind="stable")
    sv_vox_s = sv_vox[svo]
    sv_mask_s = sv_mask[svo]
    sv_isb_s = sv_isb[svo]

    cls_masks, cls_starts, cls_counts = np.unique(
        sv_mask_s, return_index=True, return_counts=True)
    j_in_cls = np.arange(nsv) - np.repeat(cls_starts, cls_counts)
    sv_core = j_in_cls % N_CORES
    sv_col = j_in_cls // N_CORES

    # ---- psum layout + per-segment packing (identical on all cores) -----
    # HW quirk: two matmuls with the same rounded tile K but different row
    # offsets hitting the same PSUM col-group crash the NEFF.  Workaround:
    # the row offset is a fixed function of (col-group, K):
    #   h=1 (K=32):  r = g;   h=2 (K=64): r = 2*(g%2);   h>=3: r = 0
    # Blocks (512 psum cols) are filled greedily with the height whose
    # destination lanes currently have the lowest cursor -> balances the
    # 4 xslab lanes and mixes K-sizes across adjacent blocks.
    C = -(-cls_counts // N_CORES)          # cols per class per core
    H = _PC[cls_masks]

    def _row_of(h, g):
        if h == 1:
            return g
        if h == 2:
            return 2 * (g % 2)
        return 0

    lane = [0, 0, 0, 0]    # xslab lane cursors
    wlane = [0, 0, 0, 0]   # weight slab lane cursors (32-col block units)
    wblocks = {}           # (mask, r) -> wcol
    cls_segs = {m: [] for m in map(int, cls_masks)}
    # seg: (cls_off, nw, psum_s, g, r, col0)

    def _walloc(mask, h, r):
        kb = (mask, r)
        if kb not in wblocks:
            wc = max(wlane[r:r + h])
            for rr in range(r, r + h):
                wlane[rr] = wc + 1
            wblocks[kb] = wc * 32
        return wblocks[kb]

    # per-height FIFOs of [class_index, taken_so_far]
    fifos = {h: [[ci, 0] for ci in np.argsort(-C, kind="stable")
                 if int(H[ci]) == h and C[ci] > 0]
             for h in (1, 2, 3, 4)}

    s = 0          # psum cursor
    while any(fifos.values()):
        g = (s % TILE_SLOTS) // PSUM_N
        space = PSUM_N - (s % PSUM_N)
        while space > 0 and any(fifos.values()):
            # pick height whose lanes have the lowest cursor
            best_h, best_key = None, None
            for h in (4, 3, 2, 1):
                if not fifos[h]:
                    continue
                r = _row_of(h, g)
                key = (max(lane[r:r + h]), -h)
                if best_key is None or key < best_key:
                    best_h, best_key = h, key
            h = best_h
            r = _row_of(h, g)
            ent = fifos[h][0]
            ci = ent[0]
            mk = int(cls_masks[ci])
            nw = min(int(C[ci]) - ent[1], space)
            col0 = max(lane[r:r + h])
            if col0 // CHUNK_COLS != (col0 + nw - 1) // CHUNK_COLS:
                col0 = ((col0 // CHUNK_COLS) + 1) * CHUNK_COLS
            for rr in range(r, r + h):
                lane[rr] = col0 + nw
            _walloc(mk, h, r)
            cls_segs[mk].append((ent[1], nw, s, g, r, col0))
            ent[1] += nw
            if ent[1] >= int(C[ci]):
                fifos[h].pop(0)
            s += nw
            space -= nw
    Q = s

    # dedicated pad weight blocks (zero weights) per row, K=32
    pad_w = [_walloc((-1, g), 1, g) for g in range(4)]

    # zero pad block so every slot of every used psum tile gets a matmul
    F0 = -(-max(lane) // 64) * 64
    if F0 % CHUNK_COLS > CHUNK_COLS - PSUM_N:
        F0 = (F0 // CHUNK_COLS + 1) * CHUNK_COLS
    pad_base = F0
    F = F0 + PSUM_N
    n_ptiles = -(-Q // TILE_SLOTS)
    nstage = -(-n_ptiles // 4)
    nchunk = -(-F // CHUNK_COLS)
    chunks = [(i * CHUNK_COLS, min((i + 1) * CHUNK_COLS, F))
              for i in range(nchunk)]

    # ---- matmul segment list per psum tile ------------------------------
    # segment: (group, c0, c1, xr, h, wr, wcol, chunk_i, xlo)
    tile_mms = [[[] for _ in range(4)] for _ in range(n_ptiles)]
    for ci in range(len(cls_masks)):
        mk = int(cls_masks[ci])
        h = int(H[ci])
        for (off, nw, ps_s, g, r, col0) in cls_segs[mk]:
            pt = ps_s // TILE_SLOTS
            c0 = ps_s % PSUM_N
            wcol = wblocks[(mk, r)]
            chunk_i = col0 // CHUNK_COLS
            xlo = col0 - chunk_i * CHUNK_COLS
            tile_mms[pt][g].append(
                (g, c0, c0 + nw, r, h, r, wcol, chunk_i, xlo))
    # pad matmuls over the zero block (zero weights x zero data)
    s = Q
    pad_ci = pad_base // CHUNK_COLS
    pad_xlo = pad_base - pad_ci * CHUNK_COLS
    while s < n_ptiles * TILE_SLOTS:
        s1 = min(n_ptiles * TILE_SLOTS, (s // PSUM_N + 1) * PSUM_N)
        pt = s // TILE_SLOTS
        g = (s % TILE_SLOTS) // PSUM_N
        c0 = s % PSUM_N
        tile_mms[pt][g].append(
            (g, c0, c0 + (s1 - s), g, 1, g, pad_w[g], pad_ci, pad_xlo))
        s = s1

    WF = max(max(wlane) * 32, 32)

    layout = {
        "F": F, "WF": WF, "Q": Q, "n_ptiles": n_ptiles, "nstage": nstage,
        "chunks": tuple(chunks),
        "tile_mms": tuple(tuple(tuple(g) for g in t) for t in tile_mms),
    }
    key = (F, WF, n_ptiles, nstage, layout["chunks"], layout["tile_mms"])

    # ---- per-sv placement (segment-aware) -------------------------------
    import ml_dtypes
    bf16 = ml_dtypes.bfloat16

    sv_r = np.zeros(nsv, np.int64)       # lane row of the sv's segment
    sv_xcol = np.zeros(nsv, np.int64)    # xslab column of the sv
    sv_slot = np.zeros(nsv, np.int64)    # psum slot of the sv
    for ci in range(len(cls_masks)):
        mk = int(cls_masks[ci])
        sl = slice(cls_starts[ci], cls_starts[ci] + cls_counts[ci])
        colc_l = sv_col[sl]
        segs = sorted(cls_segs[mk])
        offs = np.array([sg[0] for sg in segs])
        ps_ss = np.array([sg[2] for sg in segs])
        rs = np.array([sg[4] for sg in segs])
        c0s = np.array([sg[5] for sg in segs])
        si = np.searchsorted(offs, colc_l, side="right") - 1
        sv_r[sl] = rs[si]
        sv_xcol[sl] = c0s[si] + (colc_l - offs[si])
        sv_slot[sl] = ps_ss[si] + (colc_l - offs[si])

    # point -> stream-voxel (sorted index)
    inA = ((amask[vox] >> kk) & 1).astype(np.int64)
    pkey = vox * 2 + (1 - inA)
    sv_key_s = sv_vox_s * 2 + sv_isb_s
    ks = np.argsort(sv_key_s, kind="stable")
    pos = ks[np.searchsorted(sv_key_s[ks], pkey)]
    pm = sv_mask_s[pos]
    slot = _RANK[pm, kk]
    p_core = sv_core[pos]
    p_col = sv_xcol[pos]
    p_rb = sv_r[pos] + slot

    xslab = np.zeros((N_CORES, 4, 32, F), np.float32)
    xslab[p_core, p_rb, :, p_col] = xv
    xslab = xslab.reshape(N_CORES, 128, F).astype(bf16)

    # ---- weight slab (same for all cores) -------------------------------
    wslab = np.zeros((4, 32, WF), np.float32)
    for (mk, r), wcol in wblocks.items():
        if not isinstance(mk, (int, np.integer)):
            continue   # pad block stays zero
        h = int(_PC[mk])
        bits = [b for b in range(8) if (mk >> b) & 1]
        st = np.stack([W[b] for b in bits])          # [h, 32, 32]
        for s in range(h):
            wslab[r + s, :, wcol:wcol + 32] = st[s]
    wslab = wslab.reshape(128, WF).astype(bf16)

    # ---- decode metadata -------------------------------------------------
    meta = {
        "nstage": nstage, "num_out": num_out,
        "sv_core": sv_core, "sv_slot": sv_slot,
        "sv_vox": sv_vox_s, "sv_isb": sv_isb_s,
    }
    return key, layout, meta, xslab, wslab


# ------------------------------------------------------------ device program
def _build_program(key, layout):
    import concourse.tile as tile
    from concourse import bacc, mybir

    bf16 = mybir.dt.bfloat16
    f32 = mybir.dt.float32
    F, WF = layout["F"], layout["WF"]
    n_ptiles, nstage = layout["n_ptiles"], layout["nstage"]
    chunks = layout["chunks"]
    tile_mms = layout["tile_mms"]

    nc = bacc.Bacc("TRN2", target_bir_lowering=False, debug=False)
    x_d = nc.dram_tensor("xslab", [128, F], bf16, kind="ExternalInput")
    w_d = nc.dram_tensor("wslab", [128, WF], bf16, kind="ExternalInput")
    out_d = nc.dram_tensor("out_st", [nstage, 128, 2048], bf16,
                           kind="ExternalOutput")

    with tile.TileContext(nc) as tc:
        with (
            tc.tile_pool(name="w", bufs=1) as wpool,
            tc.tile_pool(name="xin", bufs=1) as xpool,
            tc.tile_pool(name="st", bufs=3) as stpool,
            tc.tile_pool(name="ps", bufs=8, space="PSUM") as pspool,
        ):
            engs = [nc.sync, nc.scalar, nc.gpsimd]
            w = wpool.tile([128, WF], bf16, tag="w")
            # weights split across the three queues, ahead of the x chunks
            wsplit = [0, -(-WF // 96) * 32, -(-WF // 96) * 64, WF]
            for i in range(3):
                lo, hi = wsplit[i], wsplit[i + 1]
                if hi > lo:
                    engs[i].dma_start(w[:, lo:hi], w_d.ap()[:, lo:hi])

            xt = []
            for i, (lo, hi) in enumerate(chunks):
                t = xpool.tile([128, hi - lo], bf16, tag=f"x{i}")
                engs[i % 3].dma_start(t[:], x_d.ap()[:, lo:hi])
                xt.append(t)

            for stage in range(nstage):
                stg = stpool.tile([128, 2048], bf16, tag="stg")
                n_valid = min(4, n_ptiles - stage * 4)
                for p in range(n_valid):
                    pt = stage * 4 + p
                    ps = pspool.tile([128, PSUM_N], f32, tag="ps")
                    # interleave issue across the 4 col-groups
                    groups = [list(g) for g in tile_mms[pt]]
                    while any(groups):
                        for g in range(4):
                            if groups[g]:
                                (gg, c0, c1, r, h, wr, wcol, ci, xlo) = \
                                    groups[g].pop(0)
                                nw = c1 - c0
                                nc.tensor.matmul(
                                    ps[32 * gg:32 * gg + 32, c0:c1],
                                    w[32 * wr:32 * (wr + h), wcol:wcol + 32],
                                    xt[ci][32 * r:32 * (r + h), xlo:xlo + nw],
                                    start=True, stop=True,
                                    tile_position=(32 * r, 32 * gg))
                    eng = nc.vector if p % 2 == 0 else nc.scalar
                    if eng is nc.vector:
                        eng.tensor_copy(stg[:, 512 * p:512 * (p + 1)], ps[:])
                    else:
                        eng.copy(stg[:, 512 * p:512 * (p + 1)], ps[:])
                engs[(stage + 2) % 3].dma_start(
                    out_d.ap()[stage][:, :512 * n_valid],
                    stg[:, :512 * n_valid])

    nc.compile()
    return nc


def _get_program(key, layout):
    if key not in _prog_cache:
        _prog_cache[key] = _build_program(key, layout)
    return _prog_cache[key]


# -------------------------------------------------------------------- decode
def _decode(results, meta):
    nstage = meta["nstage"]
    num_out = meta["num_out"]
    per_core = []
    for r in results:
        st = np.asarray(r["out_st"]).astype(np.float32)   # [nstage,128,2048]
        arr = st.reshape(nstage, 4, 32, 4, 512)           # [s, g, ch, p, col]
        arr = arr.transpose(0, 3, 1, 4, 2).reshape(-1, 32)  # (s,p,g,col),ch
        per_core.append(arr)
    stacked = np.stack(per_core)                          # [8, slots, 32]
    vals = stacked[meta["sv_core"], meta["sv_slot"]]      # [nsv, 32]
    out = np.zeros((num_out, 32), np.float32)
    isb = meta["sv_isb"].astype(bool)
    out[meta["sv_vox"][~isb]] = vals[~isb]
    np.add.at(out, meta["sv_vox"][isb], vals[isb])
    return out


# ---------------------------------------------------------------------- main
def run(x, W, k_idx, out_idx, num_out, trace=False, dt_name=DT_IN):
    from concourse.bass_utils import run_bass_kernel_spmd

    x = np.asarray(x, dtype=np.float32)
    W = np.asarray(W, dtype=np.float32)
    k_idx = np.asarray(k_idx, dtype=np.int32)
    out_idx = np.asarray(out_idx, dtype=np.int32)
    num_out = int(num_out)

    key, layout, meta, xslab, wslab = _plan(x, W, k_idx, out_idx, num_out)
    nc = _get_program(key, layout)
    in_maps = [{"xslab": np.ascontiguousarray(xslab[c]), "wslab": wslab}
               for c in range(N_CORES)]
    res = run_bass_kernel_spmd(nc, in_maps, list(range(N_CORES)), trace=trace)
    out = _decode(res.results, meta)
    return out, res


def kernel(x, W, k_idx, out_idx, num_out):
    out, _ = run(x, W, k_idx, out_idx, num_out, trace=False)
    return out
